# revision 5
# baseline (speedup 1.0000x reference)
"""MoE gating kernel for Trainium2 (8 NeuronCores, SPMD).

Computes, for x [4, 4096, 2048] f32 and W [64, 2048] f32:
    logits = x_flat @ W.T          # [16384, 64]
    top2 values/indices (sorted descending), softmax over the top-2 logits
Returns (indices int32 [16384, 2], values f32 [16384, 2]) — matching
jax.lax.top_k + softmax in the reference.

Strategy (per core, 2048 tokens = 4 blocks x 512):
  - DMA x naturally (tokens on partitions, D contiguous) — full HBM BW.
  - PE-transpose 128x128 tiles (fp32r, 1.5 cyc/row) to put D on partitions.
  - Gating matmul fp32r (1 cyc/row at N=512): W.T chunks [128d, 64e]
    stationary, x.T [128d, 512t] moving, accumulate logits [64e, 512t] PSUM.
  - PE-transpose logits back to [128t, 64e]; DVE max/max_index for top-2;
    ACT sigmoid for the 2-way softmax; tiny DMA out.
"""

import sys

for _p in ("/opt/trn_rl_repo", "/root/problem/work"):
    if _p not in sys.path:
        sys.path.insert(0, _p)

import numpy as np

import concourse.bass as bass
import concourse.mybir as mybir
from concourse.tile import TileContext
from concourse.bass_utils import run_bass_kernel_spmd

N_CORES = 8
TOKENS = 16384
D = 2048
E = 64
TOK_PER_CORE = TOKENS // N_CORES  # 2048
BLOCK = 512                       # tokens per matmul block
N_BLOCKS = TOK_PER_CORE // BLOCK  # 4
JTILES = BLOCK // 128             # 4 (128-token tiles per block)
KCHUNKS = D // 128                # 16 (contraction chunks)
NTILES = TOK_PER_CORE // 128      # 16 (128-token tiles per core)

F32R = mybir.dt.float32r
F32 = mybir.dt.float32
U32 = mybir.dt.uint32

_CACHE = {}


def _split_multi_waits(nc, max_waits=1):
    """walrus in this env supports only ONE sync wait per instruction's
    sync_info; split extras onto preceding NOPs on the same engine."""
    n = 0
    for fn in nc.m.functions:
        for bb in fn.blocks:
            out = []
            for inst in bb.instructions:
                si = inst.sync_info
                if si is not None and si.on_wait is not None and len(si.on_wait) > max_waits:
                    waits = list(si.on_wait)
                    head, tail = waits[:-max_waits], waits[-max_waits:]
                    k = 0
                    while head:
                        chunk, head = head[:max_waits], head[max_waits:]
                        out.append(mybir.InstNoOp(
                            name=f"{inst.name}-wsplit{k}",
                            engine=inst.engine, ins=[], outs=[],
                            sync_info=mybir.SyncInfo(on_wait=chunk, on_update=[]),
                        ))
                        k += 1
                        n += 1
                    inst.sync_info = mybir.SyncInfo(
                        on_wait=tail, on_update=list(si.on_update or []))
                out.append(inst)
            bb.instructions = out
    return n


def build_nc():
    nc = bass.Bass(trn_type="TRN2")
    x = nc.dram_tensor("x", [TOK_PER_CORE, D], F32R, kind="ExternalInput")
    w = nc.dram_tensor("W", [E, D], F32R, kind="ExternalInput")
    # identity matrices supplied as inputs: gpsimd memset on float32r
    # fails the walrus ISA check, so make_identity is unusable here
    ident_in = nc.dram_tensor("ident", [128, 128], F32R, kind="ExternalInput")
    ident64_in = nc.dram_tensor("ident64", [64, 64], F32, kind="ExternalInput")
    out_val = nc.dram_tensor("out_val", [128, NTILES, 2], F32, kind="ExternalOutput")
    out_idx = nc.dram_tensor("out_idx", [128, NTILES, 2], U32, kind="ExternalOutput")

    # DRAM view: token = b*512 + j*128 + p  ->  [b, p, j, d]
    x_v = x.rearrange("(b j p) d -> b p j d", b=N_BLOCKS, j=JTILES, p=128)

    with TileContext(nc) as tc:
        with (
            tc.tile_pool(name="singles", bufs=1) as singles,
            tc.tile_pool(name="xb", bufs=2) as xb_pool,
            tc.tile_pool(name="xt", bufs=17) as xt_pool,
            tc.tile_pool(name="lg", bufs=2) as lg_pool,
            tc.tile_pool(name="small", bufs=1) as small,
            tc.tile_pool(name="psT", bufs=3, space="PSUM") as psT_pool,
            tc.tile_pool(name="psL", bufs=2, space="PSUM") as psL_pool,
            tc.tile_pool(name="psS", bufs=2, space="PSUM") as psS_pool,
        ):
            ident_r = singles.tile([128, 128], F32R)
            nc.sync.dma_start(out=ident_r, in_=ident_in[:, :])
            ident_f = singles.tile([64, 64], F32)
            nc.sync.dma_start(out=ident_f, in_=ident64_in[:, :])

            # W -> SBUF, then 16 PE transposes -> WT [128d, 16c x 64e]
            w_sb = singles.tile([E, D], F32R)
            nc.sync.dma_start(out=w_sb, in_=w[:, :])
            wt = singles.tile([128, KCHUNKS * E], F32R)
            for c in range(KCHUNKS):
                ps = psS_pool.tile([128, E], F32R, tag="psS")
                nc.tensor.transpose(ps, w_sb[:, c * 128:(c + 1) * 128],
                                    ident_r[:E, :E])
                nc.vector.tensor_copy(wt[:, c * E:(c + 1) * E], ps)

            ltok = singles.tile([128, NTILES * E], F32)

            for b in range(N_BLOCKS):
                xb = xb_pool.tile([128, JTILES, D], F32R, tag="xb")
                nc.sync.dma_start(out=xb, in_=x_v[b])

                # transpose x block: 16 chunks x 4 jtiles -> xT [128d, 512t]
                xts = []
                for c in range(KCHUNKS):
                    psT = psT_pool.tile([128, BLOCK], F32R, tag="psT")
                    for j in range(JTILES):
                        nc.tensor.transpose(
                            psT[:, j * 128:(j + 1) * 128],
                            xb[:, j, c * 128:(c + 1) * 128],
                            ident_r)
                    xt = xt_pool.tile([128, BLOCK], F32R, tag="xt")
                    if c % 2 == 0:
                        nc.vector.tensor_copy(xt, psT)
                    else:
                        nc.scalar.copy(out=xt, in_=psT)
                    xts.append(xt)

                # gating matmuls: accumulate logits [64e, 512t]
                psL = psL_pool.tile([E, BLOCK], F32, tag="psL")
                for c in range(KCHUNKS):
                    nc.tensor.matmul(
                        psL, lhsT=wt[:, c * E:(c + 1) * E], rhs=xts[c],
                        start=(c == 0), stop=(c == KCHUNKS - 1))

                lg = lg_pool.tile([E, BLOCK], F32, tag="lg")
                nc.vector.tensor_copy(lg, psL)

                # transpose logits -> [128t, 64e] per 128-token tile
                for k in range(JTILES):
                    t = b * JTILES + k
                    ps2 = psS_pool.tile([128, E], F32, tag="psS")
                    nc.tensor.transpose(ps2, lg[:, k * 128:(k + 1) * 128],
                                        ident_f)
                    nc.scalar.copy(out=ltok[:, t * E:(t + 1) * E], in_=ps2)

            # top-2 per token tile
            maxb = small.tile([128, NTILES, 8], F32)
            idxb = small.tile([128, NTILES, 8], U32)
            for t in range(NTILES):
                nc.vector.max(out=maxb[:, t, :], in_=ltok[:, t * E:(t + 1) * E])
                nc.vector.max_index(out=idxb[:, t, :], in_max=maxb[:, t, :],
                                    in_values=ltok[:, t * E:(t + 1) * E])

            # softmax over top-2: p0 = sigmoid(l0-l1), p1 = sigmoid(l1-l0)
            d10 = small.tile([128, NTILES], F32)
            nc.vector.tensor_sub(d10, maxb[:, :, 1], maxb[:, :, 0])
            valb = small.tile([128, NTILES, 2], F32)
            nc.scalar.activation(valb[:, :, 1], d10,
                                 mybir.ActivationFunctionType.Sigmoid)
            nc.scalar.activation(valb[:, :, 0], d10,
                                 mybir.ActivationFunctionType.Sigmoid,
                                 scale=-1.0)
            idxo = small.tile([128, NTILES, 2], U32)
            nc.vector.tensor_copy(idxo, idxb[:, :, 0:2])

            nc.sync.dma_start(out=out_val[:, :, :], in_=valb)
            nc.sync.dma_start(out=out_idx[:, :, :], in_=idxo)

    _split_multi_waits(nc)
    return nc


def _get_nc():
    if "nc" not in _CACHE:
        _CACHE["nc"] = build_nc()
    return _CACHE["nc"]


def kernel(x: np.ndarray, W: np.ndarray, _trace=False, _tmpdir=None):
    nc = _get_nc()
    x_flat = np.ascontiguousarray(x.reshape(TOKENS, D).astype(np.float32))
    Wc = np.ascontiguousarray(W.astype(np.float32))
    ident = np.eye(128, dtype=np.float32)
    ident64 = np.eye(64, dtype=np.float32)
    in_maps = [
        {"x": x_flat[c * TOK_PER_CORE:(c + 1) * TOK_PER_CORE], "W": Wc,
         "ident": ident, "ident64": ident64}
        for c in range(N_CORES)
    ]
    res = run_bass_kernel_spmd(nc, in_maps, core_ids=list(range(N_CORES)),
                               trace=_trace, tmpdir=_tmpdir)
    _CACHE["last_result"] = res
    idx_parts, val_parts = [], []
    for c in range(N_CORES):
        r = res.results[c]
        # [128p, 16t, 2] -> token local = t*128 + p
        val_parts.append(r["out_val"].transpose(1, 0, 2).reshape(TOK_PER_CORE, 2))
        idx_parts.append(r["out_idx"].transpose(1, 0, 2).reshape(TOK_PER_CORE, 2)
                         .astype(np.int32))
    return (np.concatenate(idx_parts, 0), np.concatenate(val_parts, 0))
